# revision 20
# baseline (speedup 1.0000x reference)
"""Trainium2 Bass kernel for a Linformer transformer block (nn_Block).

Shapes (hardcoded): B=2, N=8192, C=768, H=12, D=64, K=256, HID=3072.
Sharding: 8 cores, data-parallel over tokens (2048 tokens/core, batch-major:
cores 0-3 hold batch 0, cores 4-7 batch 1). The Linformer K/V projections
reduce over the full sequence, so each core computes partials over its token
shard and a grouped AllReduce (replica groups [0-3], [4-7]) combines them.

Precision: all large GEMMs use fp8 e4m3 with DoubleRow perf mode (two
128-deep contraction chunks per instruction, 4x bf16 MAC rate). Weights are
pre-scaled by 128 on the host; the MLP weights carry a second fp8 residual
term (W ~= W8 + R8) consumed as a second accumulation chain to cancel
weight-quantization noise. Attention logits run in bf16. LayerNorm rstd is
computed as exp(-0.5*ln(var)) so the scalar engine never leaves the ln/exp
activation table during softmax; the MLP's gelu is the only other table
(activation-table reloads cost 1.3us each). Stage B is phase-major (all
attention, then all proj/LN2, then all MLP) for the same reason, with the
attention+proj result x2 parked in DRAM between phases. qT is computed
inside the AllReduce window to hide the collective's latency.
"""

import sys
sys.path.insert(0, "/opt/trn_rl_repo")

import numpy as np
import ml_dtypes

import concourse.bass as bass
import concourse.mybir as mybir
import concourse.tile as tile
from concourse import bacc
from concourse.bass_utils import run_bass_kernel_spmd
from concourse.masks import make_identity

F32 = mybir.dt.float32
BF16 = mybir.dt.bfloat16
FP8 = mybir.dt.float8e4
AF = mybir.ActivationFunctionType
ALU = mybir.AluOpType
DR = mybir.MatmulPerfMode.DoubleRow

B, N, C = 2, 8192, 768
H, K = 12, 256
D = C // H                 # 64
HID = 4 * C                # 3072
EPS = 1e-6
NCORES = 8
T = (B * N) // NCORES      # 2048 tokens per core
NT = T // 128              # 16 token tiles
NG = T // 512              # 4 token groups
CK = C // 128              # 6 contraction chunks of C
HC = HID // 128            # 24 hidden chunks
KC = K // 128              # 2 kk chunks
SCALE = float(D) ** -0.5   # 0.125

SW = 128.0                 # host-side weight scale into fp8 range
SO = 16.0                  # oT carries 16*o (via 1/16-valued ones vector)

_CACHE = {}


def _ln_stats(nc, pool, xt, tag):
    """LayerNorm stats for a (128, C) fp32 tile -> (rstd, -mu*rstd) (128,1)."""
    NSUB = 3  # 768 = 3 x 256 (BN_STATS_FMAX=512, gcd=256)
    stats = pool.tile([128, NSUB, 6], F32, tag=f"{tag}_stats")
    xv = xt.rearrange("p (j s) -> p j s", j=NSUB)
    for j in range(NSUB):
        nc.vector.bn_stats(stats[:, j, :], xv[:, j, :])
    mv = pool.tile([128, 2], F32, tag=f"{tag}_mv")
    nc.vector.bn_aggr(mv[:], stats[:])
    var = pool.tile([128, 1], F32, tag=f"{tag}_var")
    nc.vector.tensor_scalar_add(var[:], mv[:, 1:2], EPS)
    # rstd = exp(-0.5*ln(var)): keeps the ACT engine on the ln/exp table
    # (sqrt lives in a different table; each table swap costs 1.3us).
    lv = pool.tile([128, 1], F32, tag=f"{tag}_lv")
    nc.scalar.activation(lv[:], var[:], AF.Ln)
    rstd = pool.tile([128, 1], F32, tag=f"{tag}_rstd")
    nc.scalar.activation(rstd[:], lv[:], AF.Exp, scale=-0.5)
    nmr = pool.tile([128, 1], F32, tag=f"{tag}_nmr")
    nc.vector.scalar_tensor_tensor(nmr[:], mv[:, 0:1], -1.0, rstd[:],
                                   op0=ALU.mult, op1=ALU.mult)
    return rstd, nmr


def build(ln1_triv, ln2_triv, qb_zero, kvb_zero, projb_zero, fc2b_zero):
    nc = bacc.Bacc("TRN2", target_bir_lowering=False, debug=False,
                   enable_asserts=True, num_devices=NCORES)

    x_s = nc.dram_tensor("x_s", [T, C], F32, kind="ExternalInput").ap()
    qkv_w = nc.dram_tensor("qkv_w", [128, CK // 2, 2, 3 * C], FP8,
                           kind="ExternalInput").ap()
    qkv_b = nc.dram_tensor("qkv_b", [3 * C], F32, kind="ExternalInput").ap()
    Ek_s = nc.dram_tensor("Ek_s", [128, NT // 2, 2, K], FP8,
                          kind="ExternalInput").ap()
    Ev_s = nc.dram_tensor("Ev_s", [128, NT // 2, 2, K], FP8,
                          kind="ExternalInput").ap()
    proj_w = nc.dram_tensor("proj_w", [128, CK // 2, 2, C], FP8,
                            kind="ExternalInput").ap()
    proj_b = nc.dram_tensor("proj_b", [C], F32, kind="ExternalInput").ap()
    fc1_w = nc.dram_tensor("fc1_w", [128, 2, CK // 2, 2, HID], FP8,
                           kind="ExternalInput").ap()
    fc1_b = nc.dram_tensor("fc1_b", [HID], F32, kind="ExternalInput").ap()
    fc2_w = nc.dram_tensor("fc2_w", [128, 2, HC // 2, 2, C], FP8,
                           kind="ExternalInput").ap()
    fc2_b = nc.dram_tensor("fc2_b", [C], F32, kind="ExternalInput").ap()
    ln1_w = nc.dram_tensor("ln1_w", [1, C], F32, kind="ExternalInput").ap()
    ln1_b = nc.dram_tensor("ln1_b", [1, C], F32, kind="ExternalInput").ap()
    ln2_w = nc.dram_tensor("ln2_w", [1, C], F32, kind="ExternalInput").ap()
    ln2_b = nc.dram_tensor("ln2_b", [1, C], F32, kind="ExternalInput").ap()
    out = nc.dram_tensor("out", [T, C], F32, kind="ExternalOutput").ap()

    with tile.TileContext(nc) as tc:
      with tc.tile_pool(name="const", bufs=1) as constp, \
           tc.tile_pool(name="dram", bufs=1, space="DRAM") as dram:
        ident = constp.tile([128, 128], F32, tag="ident")
        make_identity(nc, ident)
        # 1/16-valued "ones": denominator = sum(e)/16, so rcb = 16/sum(e)
        # and oT carries 16*o, comfortably inside fp8's normal range.
        ones8 = constp.tile([128, 64], FP8, tag="ones8")
        nc.scalar.activation(ones8[:], ident[:, 0:64], AF.Copy,
                             bias=1.0 / SO, scale=0.0)
        nbias2 = constp.tile([128, 1], F32, tag="nbias2")
        nc.scalar.activation(nbias2[:], ident[:, 0:1], AF.Copy,
                             bias=-2.0, scale=0.0)
        qb_sb = constp.tile([128, CK], F32, tag="qb_sb")
        nc.sync.dma_start(
            qb_sb[:], qkv_b[None, 0:C].rearrange("o (m p) -> p (o m)", p=128))
        fc1b = constp.tile([128, HC], F32, tag="fc1b")
        nc.sync.dma_start(fc1b[:], fc1_b.rearrange("(m p) -> p m", p=128))

        def bcast_row(name, src_ap, width):
            row = constp.tile([1, width], F32, tag=f"{name}_row")
            nc.sync.dma_start(row[:], src_ap)
            bc = constp.tile([128, width], F32, tag=f"{name}_bc")
            nc.gpsimd.partition_broadcast(bc[:], row[:])
            return bc

        ln1w_bc = ln1b_bc = ln2w_bc = ln2b_bc = None
        kvb_bc = projb_bc = fc2b_bc = None
        if not ln1_triv:
            ln1w_bc = bcast_row("ln1w", ln1_w[:], C)
            ln1b_bc = bcast_row("ln1b", ln1_b[:], C)
        if not ln2_triv:
            ln2w_bc = bcast_row("ln2w", ln2_w[:], C)
            ln2b_bc = bcast_row("ln2b", ln2_b[:], C)
        if not kvb_zero:
            kvb_bc = bcast_row("kvb", qkv_b[None, C:3 * C], 2 * C)
        if not projb_zero:
            projb_bc = bcast_row("projb", proj_b[None, :], C)
        if not fc2b_zero:
            fc2b_bc = bcast_row("fc2b", fc2_b[None, :], C)

        ar_in = dram.tile([128, 2, 1536], F32)
        ar_out = dram.tile([128, 2, 1536], F32)
        x2d = dram.tile([NT, 128, C], F32)

        # ===== Stage A: LN1, h1T (DMA xbar), k+Ek partials; then v ========
        with tc.tile_pool(name="Apool", bufs=1) as Ap:
            qT = Ap.tile([128, CK, T], BF16, tag="qT")
            aw_ctx = tc.tile_pool(name="Aw", bufs=1)
            Aw = aw_ctx.__enter__()
            h1T = Aw.tile([128, CK // 2, 2, T], FP8, tag="h1T")
            qkvw_sb = Aw.tile([128, CK // 2, 2, 3 * C], FP8, tag="qkvw")
            nc.sync.dma_start(qkvw_sb[:], qkv_w)
            Ek_sb = Aw.tile([128, NT // 2, 2, K], FP8, tag="Ek_sb")
            nc.sync.dma_start(Ek_sb[:], Ek_s)
            Ev_sb = Aw.tile([128, NT // 2, 2, K], FP8, tag="Ev_sb")
            nc.sync.dma_start(Ev_sb[:], Ev_s)

            def kv_cvt(kvp, dst, bias_slice, wk):
                """psum k/v (scaled SW) -> fp8 true scale, optional bias."""
                if bias_slice is None:
                    nc.scalar.activation(dst, kvp[:], AF.Identity,
                                         scale=1.0 / SW)
                else:
                    tmp = wk.tile([128, C], F32, tag="kvtmp")
                    nc.vector.scalar_tensor_tensor(tmp[:], kvp[:], 1.0 / SW,
                                                   bias_slice,
                                                   op0=ALU.mult, op1=ALU.add)
                    nc.vector.tensor_copy(dst, tmp[:])

            kvkv = Aw.tile([128, 2, 2, C], FP8, tag="kvkv")
            with tc.tile_pool(name="A1", bufs=4) as wk, \
                 tc.tile_pool(name="psKV", bufs=1, space="PSUM") as psKV, \
                 tc.tile_pool(name="psK", bufs=1, space="PSUM") as psK, \
                 tc.tile_pool(name="psV", bufs=1, space="PSUM") as psV:
                # ---- single sweep: LN1 + h1T + k & v partials ----
                kacc = psK.tile([128, 1536], F32, tag="kacc")
                vacc = psV.tile([128, 1536], F32, tag="vacc")
                for i in range(NT):
                    xt = wk.tile([128, C], F32, tag="xt")
                    nc.sync.dma_start(xt[:], x_s[i * 128:(i + 1) * 128, :])
                    rstd, nmr = _ln_stats(nc, wk, xt, "ln1")
                    h1 = wk.tile([128, C], BF16, tag="h1")
                    nc.gpsimd.tensor_scalar(h1[:], xt[:], rstd[:], nmr[:],
                                            op0=ALU.mult, op1=ALU.add)
                    if ln1w_bc is not None:
                        h1f = wk.tile([128, C], F32, tag="h1f")
                        nc.vector.tensor_mul(h1f[:], h1[:], ln1w_bc[:])
                        nc.vector.tensor_add(h1f[:], h1f[:], ln1b_bc[:])
                        nc.vector.tensor_copy(h1[:], h1f[:])
                    h1Tb = wk.tile([128, CK, 128], BF16, tag="h1Tb")
                    nc.scalar.dma_start(h1Tb[:], h1[:], transpose=True)
                    nc.vector.tensor_copy(
                        h1T[:, :, :, i * 128:(i + 1) * 128].rearrange(
                            "p j s t -> p (j s) t"),
                        h1Tb[:])
                    for w, woff, acc, E_sb in ((0, C, kacc, Ek_sb),
                                               (1, 2 * C, vacc, Ev_sb)):
                        kvp = psKV.tile([128, C], F32, tag="kvp")
                        for lo, hi in ((0, 512), (512, 768)):
                            for j in range(CK // 2):
                                nc.tensor.matmul(
                                    kvp[:, lo:hi],
                                    h1T[:, j, :, i * 128:(i + 1) * 128],
                                    qkvw_sb[:, j, :, woff + lo:woff + hi],
                                    start=(j == 0), stop=(j == CK // 2 - 1),
                                    perf_mode=DR)
                        kv_cvt(kvp, kvkv[:, w, i % 2, :],
                               None if kvb_bc is None
                               else kvb_bc[:, w * C:(w + 1) * C], wk)
                        if i % 2 == 1:
                            u = i // 2
                            st = (u == 0)
                            sp = (u == NT // 2 - 1)
                            for kc in range(KC):
                                for lo, hi in ((0, 512), (512, 768)):
                                    nc.tensor.matmul(
                                        acc[:, kc * 768 + lo:kc * 768 + hi],
                                        E_sb[:, u, :, kc * 128:(kc + 1) * 128],
                                        kvkv[:, w, :, lo:hi],
                                        start=st, stop=sp, perf_mode=DR)
                for w, acc in ((0, kacc), (1, vacc)):
                    acc_sb = wk.tile([128, 1536], F32, tag="acc_sb")
                    nc.scalar.activation(acc_sb[:], acc[:], AF.Copy,
                                         scale=1.0 / SW)
                    nc.sync.dma_start(ar_in[:, w, :], acc_sb[:])

            nc.gpsimd.collective_compute(
                "AllReduce", ALU.add,
                replica_groups=[[0, 1, 2, 3], [4, 5, 6, 7]],
                ins=[ar_in.opt()], outs=[ar_out.opt()])

            # ---- qT (overlaps the AllReduce) ----
            with tc.tile_pool(name="psQ", bufs=2, space="PSUM") as psQ:
                for g in range(NG):
                    for m in range(CK):
                        qp = psQ.tile([128, 512], F32, tag="qp")
                        for j in range(CK // 2):
                            nc.tensor.matmul(
                                qp[:],
                                qkvw_sb[:, j, :, m * 128:(m + 1) * 128],
                                h1T[:, j, :, g * 512:(g + 1) * 512],
                                start=(j == 0), stop=(j == CK // 2 - 1),
                                perf_mode=DR)
                        nc.scalar.activation(
                            qT[:, m, g * 512:(g + 1) * 512], qp[:],
                            AF.Identity, scale=1.0 / SW,
                            bias=(0.0 if qb_zero else qb_sb[:, m:m + 1]))
            aw_ctx.__exit__(None, None, None)

            # ===== post-AR: kT bf16 (transposed) and v_r fp8 ===============
            with tc.tile_pool(name="kvp2", bufs=1) as kvp2:
                v_r = kvp2.tile([128, KC, C], FP8, tag="v_r")
                vf = kvp2.tile([128, 1536], F32, tag="vf")
                nc.sync.dma_start(vf[:], ar_out[:, 1, :])
                nc.scalar.activation(v_r[:].rearrange("p a b -> p (a b)"),
                                     vf[:], AF.Copy)
                kT = kvp2.tile([128, CK, K], BF16, tag="kT")
                with tc.tile_pool(name="kfp", bufs=1) as kfp:
                    kf = kfp.tile([128, 1536], F32, tag="kf")
                    nc.sync.dma_start(kf[:], ar_out[:, 0, :])
                    kfb = kfp.tile([128, 1536], BF16, tag="kfb")
                    nc.scalar.activation(kfb[:], kf[:], AF.Copy)
                    for kc in range(KC):
                        kTb = kfp.tile([128, CK, 128], BF16, tag="kTb")
                        nc.sync.dma_start(
                            kTb[:], kfb[:, kc * 768:(kc + 1) * 768],
                            transpose=True)
                        nc.gpsimd.tensor_copy(
                            kT[:, :, kc * 128:(kc + 1) * 128], kTb[:])

                # ===== Stage B (phase-major to avoid ACT table swaps) ======
                with tc.tile_pool(name="at", bufs=2) as at, \
                     tc.tile_pool(name="at4", bufs=4) as at4, \
                     tc.tile_pool(name="pj", bufs=2) as pj, \
                     tc.tile_pool(name="pj4", bufs=4) as pj4, \
                     tc.tile_pool(name="ml", bufs=2) as ml, \
                     tc.tile_pool(name="ml1", bufs=1) as ml1:
                    pw = ml1.tile([128, CK // 2, 2, C], FP8, tag="pw")
                    nc.sync.dma_start(pw[:], proj_w)
                    f1w = ml1.tile([128, 2, CK // 2, 2, HID], FP8, tag="f1w")
                    nc.sync.dma_start(f1w[:], fc1_w)
                    f2w = ml1.tile([128, 2, HC // 2, 2, C], FP8, tag="f2w")
                    nc.sync.dma_start(f2w[:], fc2_w)

                    # ---- phase 1: attention for all groups ----
                    ph1_ctx = [tc.tile_pool(name="psL", bufs=2, space="PSUM"),
                               tc.tile_pool(name="psN", bufs=2, space="PSUM"),
                               tc.tile_pool(name="psO", bufs=2, space="PSUM")]
                    psL, psN, psO = [c.__enter__() for c in ph1_ctx]
                    oTs = []
                    for g in range(NG):
                        t0 = g * 512
                        oT = at4.tile([128, CK // 2, 2, 512], FP8, tag="oT")
                        oTs.append(oT)
                        for ph in range(H // 2):
                            eTs = []
                            rcbp = at.tile([128, 512], F32, tag=f"rcb{ph % 2}")
                            dn = psN.tile([128, 512], F32, tag="dn")
                            for sub in range(2):
                                h = 2 * ph + sub
                                off = 64 * (h % 2)
                                ch = h // 2
                                eT = at.tile([128, 2, 512], FP8,
                                             tag=f"eT{sub}")
                                lg = psL.tile([128, 2, 512], F32, tag="lg")
                                for kc in range(KC):
                                    nc.tensor.matmul(
                                        lg[:, kc, :],
                                        kT[off:off + 64, ch,
                                           kc * 128:(kc + 1) * 128],
                                        qT[off:off + 64, ch, t0:t0 + 512],
                                        start=True, stop=True,
                                        tile_position=(off, 0))
                                nc.scalar.activation(
                                    eT[:], lg[:], AF.Exp,
                                    scale=SCALE, bias=nbias2[:])
                                for kc in range(KC):
                                    nc.tensor.matmul(
                                        dn[off:off + 64, :],
                                        ones8[:], eT[:, kc, :],
                                        start=(kc == 0), stop=(kc == KC - 1),
                                        tile_position=(0, off))
                                eTs.append(eT)
                            nc.vector.reciprocal(rcbp[:], dn[:])
                            # o^T for the head pair (h0 DoubleRow, h1 plain)
                            pav = psO.tile([128, 512], F32, tag="pav")
                            nc.tensor.matmul(
                                pav[0:64, :],
                                v_r[:, :, ph * 128:ph * 128 + 64],
                                eTs[0][:], start=True, stop=True, perf_mode=DR)
                            for kc in range(KC):
                                nc.tensor.matmul(
                                    pav[64:128, :],
                                    v_r[:, kc, ph * 128 + 64:ph * 128 + 128],
                                    eTs[1][:, kc, :],
                                    start=(kc == 0), stop=(kc == KC - 1),
                                    tile_position=(0, 64))
                            nc.vector.tensor_tensor(
                                oT[:, ph // 2, ph % 2, :], pav[:], rcbp[:],
                                op=ALU.mult)

                    for c in reversed(ph1_ctx):
                        c.__exit__(None, None, None)

                    # ---- phase 2: proj + residual + LN2 + h2T, all groups -
                    ph2_ctx = tc.tile_pool(name="psP", bufs=2, space="PSUM")
                    psP = ph2_ctx.__enter__()
                    h2Ts = []
                    for g in range(NG):
                        t0 = g * 512
                        oT = oTs[g]
                        h2T = pj4.tile([128, CK // 2, 2, 512], FP8, tag="h2T")
                        h2Ts.append(h2T)
                        for ms in range(4):
                            r0 = t0 + ms * 128
                            xr = pj.tile([128, C], F32, tag="xr")
                            nc.sync.dma_start(xr[:], x_s[r0:r0 + 128, :])
                            x2g = pj.tile([128, C], F32, tag="x2g")
                            for cs in range(2):
                                pp = psP.tile([128, 384], F32, tag="pp")
                                for j in range(CK // 2):
                                    nc.tensor.matmul(
                                        pp[:],
                                        oT[:, j, :, ms * 128:(ms + 1) * 128],
                                        pw[:, j, :, cs * 384:(cs + 1) * 384],
                                        start=(j == 0),
                                        stop=(j == CK // 2 - 1), perf_mode=DR)
                                nc.vector.scalar_tensor_tensor(
                                    x2g[:, cs * 384:(cs + 1) * 384],
                                    pp[:], 1.0 / (SO * SW),
                                    xr[:, cs * 384:(cs + 1) * 384],
                                    op0=ALU.mult, op1=ALU.add)
                            if projb_bc is not None:
                                nc.vector.tensor_add(x2g[:], x2g[:],
                                                     projb_bc[:])
                            nc.gpsimd.dma_start(x2d[4 * g + ms], x2g[:])
                            rstd2, nmr2 = _ln_stats(nc, pj, x2g[:],
                                                    f"ln2_{ms % 2}")
                            h2 = pj.tile([128, C], BF16, tag="h2")
                            nc.gpsimd.tensor_scalar(h2[:], x2g[:],
                                                    rstd2[:], nmr2[:],
                                                    op0=ALU.mult, op1=ALU.add)
                            if ln2w_bc is not None:
                                h2f = pj.tile([128, C], F32, tag="h2f")
                                nc.vector.tensor_mul(h2f[:], h2[:], ln2w_bc[:])
                                nc.vector.tensor_add(h2f[:], h2f[:],
                                                     ln2b_bc[:])
                                nc.vector.tensor_copy(h2[:], h2f[:])
                            h2Tb = pj.tile([128, CK, 128], BF16, tag="h2Tb")
                            nc.scalar.dma_start(h2Tb[:], h2[:], transpose=True)
                            nc.vector.tensor_copy(
                                h2T[:, :, :, ms * 128:(ms + 1) * 128].rearrange(
                                    "p j s t -> p (j s) t"),
                                h2Tb[:])

                    ph2_ctx.__exit__(None, None, None)
                    ph3_ctx = tc.tile_pool(name="psF", bufs=4, space="PSUM")
                    psF = ph3_ctx.__enter__()
                    # ---- phase 3: MLP for all groups (fp8 DR, W8+R8) ----
                    # Reversed group order: MLP for the first-emitted group
                    # then waits on the LAST h2T, so no gelu issues while
                    # phase-2 ln/exp ops are still pending (ACT table swaps).
                    for g in reversed(range(NG)):
                        t0 = g * 512
                        h2T = h2Ts[g]
                        gT = ml.tile([128, HC // 2, 2, 512], FP8, tag="gT")
                        for hc in range(HC):
                            fp = psF.tile([128, 512], F32, tag="fp")
                            for t in range(2):
                                for j in range(CK // 2):
                                    nc.tensor.matmul(
                                        fp[:],
                                        f1w[:, t, j, :,
                                            hc * 128:(hc + 1) * 128],
                                        h2T[:, j, :, :],
                                        start=(t == 0 and j == 0),
                                        stop=(t == 1 and j == CK // 2 - 1),
                                        perf_mode=DR)
                            nc.scalar.activation(gT[:, hc // 2, hc % 2, :],
                                                 fp[:], AF.Gelu,
                                                 scale=1.0 / SW,
                                                 bias=fc1b[:, hc:hc + 1])
                        for cs in range(2):
                            for ms in range(4):
                                r0 = t0 + ms * 128
                                op = psF.tile([128, 512], F32, tag="fp")
                                for t in range(2):
                                    for hp in range(HC // 2):
                                        nc.tensor.matmul(
                                            op[:, 0:384],
                                            gT[:, hp, :,
                                               ms * 128:(ms + 1) * 128],
                                            f2w[:, t, hp, :,
                                                cs * 384:(cs + 1) * 384],
                                            start=(t == 0 and hp == 0),
                                            stop=(t == 1 and
                                                  hp == HC // 2 - 1),
                                            perf_mode=DR)
                                xr2 = ml.tile([128, 384], F32, tag="xr2")
                                nc.gpsimd.dma_start(
                                    xr2[:],
                                    x2d[4 * g + ms, :,
                                        cs * 384:(cs + 1) * 384])
                                oth = ml.tile([128, 384], F32, tag="oth")
                                nc.vector.scalar_tensor_tensor(
                                    oth[:], op[:, 0:384], 1.0 / SW, xr2[:],
                                    op0=ALU.mult, op1=ALU.add)
                                if fc2b_bc is not None:
                                    nc.vector.tensor_add(
                                        oth[:], oth[:],
                                        fc2b_bc[:, cs * 384:(cs + 1) * 384])
                                nc.scalar.dma_start(
                                    out[r0:r0 + 128, cs * 384:(cs + 1) * 384],
                                    oth[:])
                    ph3_ctx.__exit__(None, None, None)

    nc.compile()
    return nc


def _to_fp8(a):
    return np.ascontiguousarray(a.astype(ml_dtypes.float8_e4m3))


def _pair4(r):
    """[C_in, C_out] -> [128, C_in/256, 2, C_out] (paired contraction)."""
    ci, co = r.shape
    return np.ascontiguousarray(
        r.reshape(ci // 256, 2, 128, co).transpose(2, 0, 1, 3))


def _w_fp8(w):
    return _to_fp8(_pair4(w * SW))


def _w_fp8_split(w):
    """W*SW ~= W8 + R8 (same scale); returns [128, 2, ci/256, 2, co] fp8."""
    ws = w * SW
    w8 = ws.astype(ml_dtypes.float8_e4m3)
    r8 = (ws - w8.astype(np.float32)).astype(ml_dtypes.float8_e4m3)
    return np.ascontiguousarray(
        np.stack([_pair4(w8.astype(np.float32)),
                  _pair4(r8.astype(np.float32))], axis=1).astype(
                      ml_dtypes.float8_e4m3))


def kernel(**inputs):
    x = np.ascontiguousarray(np.asarray(inputs["x"], dtype=np.float32))
    qkv_w = np.asarray(inputs["qkv_w"], dtype=np.float32)
    qkv_b = np.ascontiguousarray(np.asarray(inputs["qkv_b"], dtype=np.float32))
    Ek = np.asarray(inputs["Ek"], dtype=np.float32)
    Ev = np.asarray(inputs["Ev"], dtype=np.float32)
    proj_w = np.asarray(inputs["proj_w"], dtype=np.float32)
    proj_b = np.ascontiguousarray(np.asarray(inputs["proj_b"], dtype=np.float32))
    fc1_w = np.asarray(inputs["fc1_w"], dtype=np.float32)
    fc1_b = np.ascontiguousarray(np.asarray(inputs["fc1_b"], dtype=np.float32))
    fc2_w = np.asarray(inputs["fc2_w"], dtype=np.float32)
    fc2_b = np.ascontiguousarray(np.asarray(inputs["fc2_b"], dtype=np.float32))
    ln1_w = np.asarray(inputs["ln1_w"], dtype=np.float32)
    ln1_b = np.asarray(inputs["ln1_b"], dtype=np.float32)
    ln2_w = np.asarray(inputs["ln2_w"], dtype=np.float32)
    ln2_b = np.asarray(inputs["ln2_b"], dtype=np.float32)

    ln1_triv = bool(np.all(ln1_w == 1.0) and np.all(ln1_b == 0.0))
    ln2_triv = bool(np.all(ln2_w == 1.0) and np.all(ln2_b == 0.0))
    qb_zero = bool(np.all(qkv_b[0:C] == 0.0))
    kvb_zero = bool(np.all(qkv_b[C:] == 0.0))
    projb_zero = bool(np.all(proj_b == 0.0))
    fc2b_zero = bool(np.all(fc2_b == 0.0))

    key = (ln1_triv, ln2_triv, qb_zero, kvb_zero, projb_zero, fc2b_zero)
    if key not in _CACHE:
        _CACHE[key] = build(*key)
    nc = _CACHE[key]

    qkv_w8 = _w_fp8(qkv_w)
    proj_w8 = _w_fp8(proj_w)
    fc1_w8 = _w_fp8_split(fc1_w)
    fc2_w8 = _w_fp8_split(fc2_w)

    def ek_prep(E, pos0):
        e = (E[pos0:pos0 + T] * SW).reshape(NT // 2, 2, 128, K)
        return _to_fp8(np.ascontiguousarray(e.transpose(2, 0, 1, 3)))

    xf = x.reshape(B * N, C)
    in_maps = []
    for c in range(NCORES):
        pos0 = (c % 4) * T
        in_maps.append({
            "x_s": np.ascontiguousarray(xf[c * T:(c + 1) * T]),
            "qkv_w": qkv_w8,
            "qkv_b": qkv_b,
            "Ek_s": ek_prep(Ek, pos0),
            "Ev_s": ek_prep(Ev, pos0),
            "proj_w": proj_w8,
            "proj_b": proj_b,
            "fc1_w": fc1_w8,
            "fc1_b": fc1_b,
            "fc2_w": fc2_w8,
            "fc2_b": fc2_b,
            "ln1_w": np.ascontiguousarray(ln1_w.reshape(1, C)),
            "ln1_b": np.ascontiguousarray(ln1_b.reshape(1, C)),
            "ln2_w": np.ascontiguousarray(ln2_w.reshape(1, C)),
            "ln2_b": np.ascontiguousarray(ln2_b.reshape(1, C)),
        })

    import os
    trace = bool(os.environ.get("NN_BLOCK_TRACE"))
    res = run_bass_kernel_spmd(nc, in_maps, core_ids=list(range(NCORES)),
                               trace=trace)
    global LAST_RESULT
    LAST_RESULT = res
    outs = np.concatenate([res.results[c]["out"] for c in range(NCORES)],
                          axis=0)
    return outs.reshape(B, N, C)


LAST_RESULT = None
